# revision 1
# baseline (speedup 1.0000x reference)
"""Trainium2 Bass kernel for nn_DynamicRangeCompressor.

Input : audio [16, 1, 2097152] f32 (+ scalar params threshold/ratio/makeup/
        attack_time/release_time as [1] arrays).
Output: [16, 1, 2097152] f32.

Sharding: pure data parallel - 2 batch rows per core across 8 NeuronCores.

Algorithm restructuring (validated vs reference to ~3e-7 rel err):
- Work in natural-log units (U = dB * ln10/20 + makeup_nat) so Ln/Exp replace
  log10/10**x and all scale factors fold away.
- linear_downsample(DS=16) == 0.5*(g[16i+7]+g[16i+8]): only 2/16 gain taps.
- The attack/release one-pole recurrence is branch-linearized: the coefficient
  is chosen by comparing gd[t] >= gd[t-1] instead of gd[t] >= y[t-1]. Since
  the coefficients are ~5e-5 the state tracks the target to ~3e-3 dB and the
  substitution changes the output by <1e-6 dB.  The scan becomes a LINEAR
  first-order IIR y = c[t]*y + b[t], which runs at stream rate on the DVE via
  tensor_tensor_scan(mult, add).
- Partition-parallel scan: each of 128 partitions scans its own time segment.
  Audio chunks are DMA'd with a (W+1)-frame overlapped window per partition,
  so each segment computes its own warmup coefficients locally and converges
  ((5.5e-5)^W contraction) - no cross-partition marshalling on the critical
  path.
- Hann overlap-add upsample == per-frame lerp: L[16q+r] = U[q] + dU[q]*w0[r],
  emitted as 16 strided scalar_tensor_tensor ops.  dU's cross-segment last
  column comes from a partition-shift DMA within the chunk; only partition
  127 (whose successor lives in the next chunk) is patched afterwards - 16
  samples per channel rewritten in SBUF and re-sent over a tiny DMA.
- out = audio * exp(L)  (drops reference's sign(a)*1e-8 term: |err| <= 1.5e-8).
"""
import os
import sys

for _p in ("/opt/trn_rl_repo", "/opt/pypackages"):
    if _p not in sys.path and os.path.isdir(_p):
        sys.path.append(_p)

import math
import numpy as np

import concourse.bass as bass
import concourse.tile as tile
from bass_rust import add_dep_helper
from concourse import bacc, mybir
from concourse.ap import AP as RawAP
from concourse.bass_utils import run_bass_kernel_spmd

# problem constants (hardcoded per spec)
B_TOTAL = 16
T = 2097152
N_CORES = 8
NCH = 2               # batch rows per core
P = 128               # SBUF partitions
FD = T // P           # 16384 free-dim samples per partition per channel
MS = [4096, 4096, 4096, 4096]   # per-chunk samples/partition/channel
assert sum(MS) == FD
S = len(MS)
W = 2                 # scan warmup frames
OVF = W + 1           # overlapped warmup frames loaded with each partition
OVS = 16 * OVF        # ... in samples

F32 = mybir.dt.float32
OP = mybir.AluOpType
AF = mybir.ActivationFunctionType

LAST_RESULTS = None   # stashed BassKernelResults for test harness introspection

# Pin all activations to the one table set that contains Abs/Ln/Relu/Exp/
# Identity together (natural_log_exp_and_others); the default greedy set
# selection alternates between two sets and reloads tables 7x per run.
import concourse.bacc as _bacc_mod
from concourse.hw_specs import get_activation_tables as _real_gat


def _gat_pinned(arch):
    real = _real_gat(arch)
    return {name: (fns if name == "natural_log_exp_and_others" else set())
            for name, fns in real.items()}


_bacc_mod.get_activation_tables = _gat_pinned


def _build(thr, ratio, makeup, at, rt):
    ln10_20 = math.log(10.0) / 20.0
    thr_nat = float(np.float32(thr * ln10_20))
    mk_nat = float(np.float32(makeup * ln10_20))
    gscale = float(np.float32(-(1.0 - 1.0 / ratio) / 2.0))   # -0.375
    at = float(np.float32(at))
    rt = float(np.float32(rt))
    w0 = [float(0.5 * (1.0 - math.cos(2.0 * math.pi * r / 32.0))) for r in range(16)]

    nc = bacc.Bacc("TRN2", target_bir_lowering=False, debug=False)
    audio = nc.dram_tensor("audio", [NCH, T], F32, kind="ExternalInput")
    out = nc.dram_tensor("out", [NCH, T], F32, kind="ExternalOutput")

    OFF = [sum(MS[:i]) * P for i in range(S)]   # chunk start sample (per ch)

    with tile.TileContext(nc) as tc:
        with tc.tile_pool(name="aud", bufs=3) as pa, \
             tc.tile_pool(name="big", bufs=2) as pb, \
             tc.tile_pool(name="fr", bufs=2) as pf, \
             tc.tile_pool(name="consts", bufs=1) as pc:

            bias_eps = pc.tile([P, 1], F32, tag="bias_eps")
            bias_nthr = pc.tile([P, 1], F32, tag="bias_nthr")
            bias_mkh = pc.tile([P, 1], F32, tag="bias_mkh")
            bias_rt = pc.tile([P, 1], F32, tag="bias_rt")
            bias_omrt = pc.tile([P, 1], F32, tag="bias_omrt")
            nc.vector.memset(bias_eps[:], 1e-8)
            nc.vector.memset(bias_nthr[:], -thr_nat)
            nc.vector.memset(bias_mkh[:], 0.5 * mk_nat)
            nc.vector.memset(bias_rt[:], rt)
            nc.vector.memset(bias_omrt[:], 1.0 - rt)
            w0t = pc.tile([P, 16], F32, tag="w0t")
            for r in range(16):
                nc.vector.memset(w0t[:, r:r + 1], w0[r])

            st = [{} for _ in range(S)]  # per-chunk tiles

            def dma_in(s):
                d = st[s]
                M = MS[s]
                MO = M + OVS
                A = pa.tile([P, 2 * MO], F32, tag="A")
                av = A[:].rearrange("p (c mo) -> p c mo", c=2)
                d["A"] = A
                if s == 0:
                    # real samples for all partitions; warmup window separately
                    # (partition 0 has no preceding audio: zero-fill its warmup)
                    nc.sync.dma_start(
                        out=av[:, 0, OVS:MO],
                        in_=RawAP(audio, 0, [[M, P], [1, M]]))
                    nc.scalar.dma_start(
                        out=av[:, 1, OVS:MO],
                        in_=RawAP(audio, T, [[M, P], [1, M]]))
                    nc.sync.dma_start(
                        out=av[1:P, 0, 0:OVS],
                        in_=RawAP(audio, M - OVS, [[M, P - 1], [1, OVS]]))
                    nc.scalar.dma_start(
                        out=av[1:P, 1, 0:OVS],
                        in_=RawAP(audio, T + M - OVS, [[M, P - 1], [1, OVS]]))
                    nc.vector.memset(av[0:1, :, 0:OVS], 0.0)
                else:
                    nc.sync.dma_start(
                        out=av[:, 0],
                        in_=RawAP(audio, OFF[s] - OVS, [[M, P], [1, MO]]))
                    nc.scalar.dma_start(
                        out=av[:, 1],
                        in_=RawAP(audio, T + OFF[s] - OVS, [[M, P], [1, MO]]))

            def frame_act(s):
                d = st[s]
                M = MS[s]
                G = M // 16
                GW = G + W
                A = d["A"]
                # tap pairs (16g+7, 16g+8) for frames [-(W+1) .. G)
                tp = pf.tile([P, 2 * (GW + 1) * 2], F32, tag="tp")
                tpv = tp[:].rearrange("p (c f two) -> p c f two", c=2, two=2)
                apv = A[:].rearrange("p (c f six) -> p c f six", c=2, six=16)
                half = (GW + 1) * 2
                if s == 0:
                    # per channel so ch0's chain overlaps ch1's inbound DMA
                    for c in range(2):
                        nc.scalar.activation(tpv[:, c:c + 1], apv[:, c:c + 1, :, 7:9],
                                             AF.Abs)
                        nc.scalar.activation(tp[:, c * half:(c + 1) * half],
                                             tp[:, c * half:(c + 1) * half],
                                             AF.Ln, bias=bias_eps[:])
                        relu_i = nc.scalar.activation(
                            tp[:, c * half:(c + 1) * half],
                            tp[:, c * half:(c + 1) * half],
                            AF.Relu, bias=bias_nthr[:])
                else:
                    nc.scalar.activation(tpv[:], apv[:, :, :, 7:9], AF.Abs)
                    nc.scalar.activation(tp[:], tp[:], AF.Ln, bias=bias_eps[:])
                    relu_i = nc.scalar.activation(tp[:], tp[:], AF.Relu,
                                                  bias=bias_nthr[:])
                d["tp"] = tp
                d["relu_i"] = relu_i.ins

            def prep_scan(s):
                d = st[s]
                M = MS[s]
                G = M // 16
                GW = G + W
                GW1 = GW + 1
                MO = M + OVS
                A = d["A"]
                tp = d["tp"]
                tpv = tp[:].rearrange("p (c f two) -> p c f two", c=2, two=2)
                # gd = gscale*(t7+t8) + mk
                gdf = pf.tile([P, 2 * (GW + 1)], F32, tag="gdf")
                gv = gdf[:].rearrange("p (c f) -> p c f", c=2)
                nc.vector.tensor_tensor(out=gv[:], in0=tpv[:, :, :, 0],
                                        in1=tpv[:, :, :, 1], op=OP.add)
                nc.vector.tensor_scalar(out=gdf[:], in0=gdf[:], scalar1=gscale,
                                        scalar2=mk_nat, op0=OP.mult, op1=OP.add)
                # branch mask m for frames [-W .. G)
                mt = pf.tile([P, 2 * GW], F32, tag="tp")
                mv = mt[:].rearrange("p (c g) -> p c g", c=2)
                nc.vector.tensor_tensor(out=mv[:], in0=gv[:, :, 1:GW + 1],
                                        in1=gv[:, :, 0:GW], op=OP.is_ge)
                # coefficient arrays [c_ch0 | c_ch1 | b_ch0 | b_ch1], each GW
                cb = pf.tile([P, 4 * GW], F32, tag="cb")
                cbv = cb[:].rearrange("p (h gw) -> p h gw", h=4)
                nc.scalar.activation(cbv[:, 0:2, :], mv[:], AF.Identity,
                                     bias=bias_rt[:], scale=at - rt)
                nc.scalar.activation(mt[:], mt[:], AF.Identity,
                                     bias=bias_omrt[:], scale=rt - at)
                nc.vector.tensor_tensor(out=cbv[:, 2:4, :], in0=mv[:],
                                        in1=gv[:, :, 1:GW + 1], op=OP.mult)
                if s == 0:
                    # channel start: hold state at the first real frame (exact)
                    nc.vector.memset(cbv[0:1, 0:2, 0:W], 1.0)
                    nc.vector.memset(cbv[0:1, 2:4, 0:W], 0.0)
                # the scan: one linear IIR per partition per channel.
                # U layout per channel: [W warmup | G real | 1 next-seg-first]
                U = pf.tile([P, 2 * GW1], F32, tag="U")
                for c in range(2):
                    nc.vector.tensor_tensor_scan(
                        out=U[:, c * GW1:c * GW1 + GW],
                        data0=cb[:, c * GW:(c + 1) * GW],
                        data1=cb[:, (2 + c) * GW:(3 + c) * GW],
                        initial=gv[:, c, W + 1:W + 2], op0=OP.mult, op1=OP.add)
                # next-segment first U: partitions 0-126 from this chunk;
                # partition 127's successor lives in chunk s+1 (patched later)
                nc.vector.memset(U[:, GW::GW1], 0.0)
                nc.sync.dma_start(out=U[0:P - 1, GW::GW1], in_=U[1:P, W::GW1])
                if s == S - 1:
                    nc.sync.dma_start(out=U[P - 1:P, GW::GW1],
                                      in_=U[P - 1:P, W + G - 1::GW1])
                d["U"] = U

            def lerp(s):
                d = st[s]
                M = MS[s]
                G = M // 16
                GW1 = G + W + 1
                MO = M + OVS
                U, A = d["U"], d["A"]
                uv = U[:].rearrange("p (c gw1) -> p c gw1", c=2)
                du = pf.tile([P, 2 * G], F32, tag="du")
                dv = du[:].rearrange("p (c g) -> p c g", c=2)
                nc.vector.tensor_tensor(out=dv[:], in0=uv[:, :, W + 1:W + G + 1],
                                        in1=uv[:, :, W:W + G], op=OP.subtract)
                # upsample lerp: L[p, c, 16g+r] = U[g] + dU[g]*w0[r]
                L = pb.tile([P, 2 * M], F32, tag="L")
                lv = L[:].rearrange("p (c m) -> p c m", c=2)
                nc.vector.tensor_copy(lv[:, :, 0::16], uv[:, :, W:W + G])
                for r in range(1, 16):
                    nc.vector.scalar_tensor_tensor(
                        out=lv[:, :, r::16], in0=dv[:], scalar=w0[r],
                        in1=uv[:, :, W:W + G], op0=OP.mult, op1=OP.add)
                av = A[:].rearrange("p (c mo) -> p c mo", c=2)
                asl = pf.tile([P, 32], F32, tag="asl")
                nc.vector.tensor_copy(
                    asl[:].rearrange("p (c r) -> p c r", c=2),
                    av[:, :, MO - 16:MO])
                d["asl"] = asl
                d["L"] = L

            def expmult(s):
                d = st[s]
                M = MS[s]
                MO = M + OVS
                A, L = d["A"], d["L"]
                av = A[:].rearrange("p (c mo) -> p c mo", c=2)
                nq = 4 if s == S - 1 else 2
                for c in range(2):
                    for qq in range(nq):
                        lo = c * M + qq * M // nq
                        hi = c * M + (qq + 1) * M // nq
                        alo = OVS + qq * M // nq
                        ahi = OVS + (qq + 1) * M // nq
                        nc.scalar.activation(L[:, lo:hi], L[:, lo:hi], AF.Exp)
                        nc.vector.tensor_tensor(out=L[:, lo:hi],
                                                in0=av[:, c, alo:ahi],
                                                in1=L[:, lo:hi], op=OP.mult)
                nc.sync.dma_start(
                    out=out[0:1, OFF[s]:OFF[s] + P * M].rearrange(
                        "one (p m) -> (one p) m", p=P),
                    in_=L[:, 0:M])
                nc.scalar.dma_start(
                    out=out[1:2, OFF[s]:OFF[s] + P * M].rearrange(
                        "one (p m) -> (one p) m", p=P),
                    in_=L[:, M:2 * M])

            def patch127(s):
                # partition 127's last frame needs chunk s+1's first U value.
                # Engines can't address a lone partition 127, so compute the
                # patch on all partitions into scratch and DMA out just row 127.
                # Deprioritized so its ACT round-trip never blocks the next
                # chunk's bulk DVE work in the engine streams.
                stk = tc.high_priority(offset=-1000000)
                stk.__enter__()
                d = st[s]
                M = MS[s]
                G = M // 16
                GW1 = G + W + 1
                MO = M + OVS
                U, asl = d["U"], d["asl"]
                q = P - 1
                unx = pf.tile([P, 2], F32, tag="unx")
                GW1n = MS[s + 1] // 16 + W + 1
                nc.vector.memset(unx[:], 0.0)
                nc.sync.dma_start(out=unx[q:q + 1, 0:2],
                                  in_=st[s + 1]["U"][0:1, W::GW1n])
                dul = pf.tile([P, 2], F32, tag="dul")
                nc.vector.tensor_tensor(out=dul[:], in0=unx[:],
                                        in1=U[:, W + G - 1::GW1], op=OP.subtract)
                uvq = U[:].rearrange("p (c gw1) -> p c gw1", c=2)
                ls = pf.tile([P, 32], F32, tag="ls")
                for c in range(2):
                    sl = ls[:, c * 16:(c + 1) * 16]
                    nc.vector.tensor_scalar(
                        out=sl, in0=w0t[:], scalar1=dul[:, c:c + 1],
                        scalar2=uvq[:, c, W + G - 1:W + G],
                        op0=OP.mult, op1=OP.add)
                    nc.scalar.activation(sl, sl, AF.Exp)
                    nc.vector.tensor_tensor(out=sl,
                                            in0=asl[:, c * 16:(c + 1) * 16],
                                            in1=sl, op=OP.mult)
                    nc.sync.dma_start(
                        out=out[c:c + 1,
                                OFF[s] + (q + 1) * M - 16:OFF[s] + (q + 1) * M],
                        in_=ls[q:q + 1, c * 16:(c + 1) * 16])
                stk.__exit__(None, None, None)

            dma_in(0)
            dma_in(1)
            frame_act(0)
            prep_scan(0)
            for s in range(S):
                lerp(s)
                if s + 2 < S:
                    dma_in(s + 2)
                if s + 1 < S:
                    frame_act(s + 1)
                    prep_scan(s + 1)
                expmult(s)
                if s > 0:
                    patch127(s - 1)

    nc.compile()
    return nc


def kernel(audio, threshold, ratio, makeup, attack_time, release_time):
    global LAST_RESULTS
    a = np.asarray(audio, dtype=np.float32)
    B, C, Tin = a.shape
    assert (B, C, Tin) == (B_TOTAL, 1, T), (B, C, Tin)
    thr = float(np.asarray(threshold).ravel()[0])
    rat = float(np.asarray(ratio).ravel()[0])
    mk = float(np.asarray(makeup).ravel()[0])
    at = float(np.asarray(attack_time).ravel()[0])
    rt = float(np.asarray(release_time).ravel()[0])

    nc = _build(thr, rat, mk, at, rt)

    flat = a.reshape(B_TOTAL, T)
    in_maps = [{"audio": np.ascontiguousarray(flat[i * NCH:(i + 1) * NCH])}
               for i in range(N_CORES)]
    res = run_bass_kernel_spmd(nc, in_maps, list(range(N_CORES)))
    LAST_RESULTS = res
    outp = np.concatenate([res.results[i]["out"] for i in range(N_CORES)], axis=0)
    return outp.reshape(B_TOTAL, 1, T).astype(np.float32)

